# revision 10
# baseline (speedup 1.0000x reference)
"""CNN attention (nn_CNNAttention_77979426226593) Trainium2 Bass kernel.

Data-parallel over batch: B=16 images -> 8 NeuronCores, 2 images per core.
Each core holds the full (small) conv1x1 weights and computes its local
N x N attention (N = H*W = 4096) independently.

Per image (C=256, N=4096, CQK=32):
  q = wq @ x + bq            [32, N]
  k = wk @ x + bk            [32, N]
  vt = x^T @ wv^T + bv       [N, 256]   (V transposed: needed as matmul lhsT)
  T[n, m] = k_n . q_m        (scores, transposed layout -> no transposes)
  E = exp(T)                 (softmax without max-subtraction: logits are
                              small by construction, exp fits fp32 easily)
  U[c, m] = sum_n vt[n, c] * E[n, m]
  d[m]    = sum_n E[n, m]    (ones-row matmul)
  out[c, m] = gamma * U[c, m] / d[m] + x[c, m]

Matmuls run in bf16 (measured ~3x faster per matmul than float32r and ~2.7x
faster than fp32 on this toolchain); accumulation is fp32 in PSUM, softmax
and normalization are fp32. The residual term x is added from a separate
fp32 copy, so when gamma == 0 the output equals the input bit-exactly.

The attention inner loop works on chunk PAIRS (one [128, 2, 512] PSUM score
tile -> a single 1024-wide exp on ScalarE) and is software-pipelined: the
score matmuls of pair g are issued before exp of pair g-1 and the U/d
matmuls of pair g-2, so the in-order PE queue never waits on ScalarE.
"""

import numpy as np

B, C, H, W = 16, 256, 64, 64
N = H * W          # 4096
CQK = 32
NCORES = 8
BPC = B // NCORES  # batches per core

MT = 512           # m tile (attention output columns per PSUM tile)
NMT = N // MT      # 8
NCH = N // 128     # 32 n-chunks (contraction for U)
NPAIR = NCH // 2   # 16 chunk pairs


def _build_nc(repeat=1):
    import contextlib
    import concourse.bacc as bacc
    import concourse.mybir as mybir
    import concourse.tile as tile
    import concourse.bass as bass

    f32 = mybir.dt.float32
    bf16 = mybir.dt.bfloat16
    AF = mybir.ActivationFunctionType
    OP = mybir.AluOpType

    nc = bacc.Bacc("TRN2", target_bir_lowering=False, debug=False,
                   num_devices=NCORES)

    xb_d = nc.dram_tensor("xb", [BPC, C, N], bf16, kind="ExternalInput")
    xf_d = nc.dram_tensor("xf", [BPC, C, N], f32, kind="ExternalInput")
    wqT_d = nc.dram_tensor("wqT", [C, CQK], bf16, kind="ExternalInput")
    wkT_d = nc.dram_tensor("wkT", [C, CQK], bf16, kind="ExternalInput")
    wvT_d = nc.dram_tensor("wvT", [C, C], bf16, kind="ExternalInput")
    bq_d = nc.dram_tensor("bq", [CQK], f32, kind="ExternalInput")
    bk_d = nc.dram_tensor("bk", [CQK], f32, kind="ExternalInput")
    bv_d = nc.dram_tensor("bv", [C], f32, kind="ExternalInput")
    gamma_d = nc.dram_tensor("gamma", [1], f32, kind="ExternalInput")
    ones_d = nc.dram_tensor("ones", [1], bf16, kind="ExternalInput")
    out_d = nc.dram_tensor("out", [BPC, C, N], f32, kind="ExternalOutput")

    def bcast_ap(handle, parts, free):
        # DRAM source AP replicated across `parts` partitions (step 0)
        return bass.AP(tensor=handle, offset=0, ap=[[0, parts], [1, free]])

    with tile.TileContext(nc) as tc:
        ctx = contextlib.ExitStack()
        with ctx:
            singles = ctx.enter_context(tc.tile_pool(name="singles", bufs=1))
            xpool = ctx.enter_context(tc.tile_pool(name="xpool", bufs=2))
            qkpool = ctx.enter_context(tc.tile_pool(name="qkpool", bufs=2))
            vtpool = ctx.enter_context(tc.tile_pool(name="vtpool", bufs=2))
            epool = ctx.enter_context(tc.tile_pool(name="epool", bufs=6))
            opool = ctx.enter_context(tc.tile_pool(name="opool", bufs=4))
            xrpool = ctx.enter_context(tc.tile_pool(name="xrpool", bufs=4))
            rpool = ctx.enter_context(tc.tile_pool(name="rpool", bufs=2))

            # --- constants / weights (once) ---
            wqT = singles.tile([C // 2, 2, CQK], bf16, tag="wqT")
            nc.gpsimd.dma_start(out=wqT, in_=wqT_d.ap().rearrange(
                "(t p) o -> p t o", p=128))
            wkT = singles.tile([C // 2, 2, CQK], bf16, tag="wkT")
            nc.gpsimd.dma_start(out=wkT, in_=wkT_d.ap().rearrange(
                "(t p) o -> p t o", p=128))
            wvT = singles.tile([C // 2, 2, C], bf16, tag="wvT")
            nc.gpsimd.dma_start(out=wvT, in_=wvT_d.ap().rearrange(
                "(t p) o -> p t o", p=128))
            bq_sb = singles.tile([CQK, 1], f32, tag="bq")
            nc.gpsimd.dma_start(out=bq_sb, in_=bq_d.ap())
            bk_sb = singles.tile([CQK, 1], f32, tag="bk")
            nc.gpsimd.dma_start(out=bk_sb, in_=bk_d.ap())
            bv_row = singles.tile([128, C], f32, tag="bvrow")
            nc.gpsimd.dma_start(out=bv_row, in_=bcast_ap(bv_d, 128, C))
            gamma_b = singles.tile([128, 1], f32, tag="gamma")
            nc.gpsimd.dma_start(out=gamma_b, in_=bcast_ap(gamma_d, 128, 1))
            ones_k = singles.tile([128, 1], bf16, tag="ones_k")
            nc.gpsimd.dma_start(out=ones_k, in_=bcast_ap(ones_d, 128, 1))

            def body():
                for b in range(BPC):
                    # --- load x (bf16 compute copy) ---
                    xt = [xpool.tile([128, N], bf16, tag=f"x{h}",
                                     name=f"xt{h}_{b}") for h in range(2)]
                    for h in range(2):
                        nc.gpsimd.dma_start(
                            out=xt[h], in_=xb_d[b, 128 * h:128 * (h + 1), :])

                    q_sb = qkpool.tile([CQK, N], bf16, tag="q")
                    k_sb = qkpool.tile([CQK, N], bf16, tag="k")
                    vt_sb = vtpool.tile([128, NCH, C], bf16, tag="vt")

                    # --- projections ---
                    with tc.tile_pool(name="ppsum", bufs=2, space="PSUM") as pp, \
                         tc.tile_pool(name="vpsum", bufs=2, space="PSUM") as vp_:
                        for nt in range(NMT):
                            ns = slice(nt * MT, (nt + 1) * MT)
                            qp = pp.tile([CQK, MT], f32, tag="qp")
                            for h in range(2):
                                nc.tensor.matmul(qp, wqT[:, h, :], xt[h][:, ns],
                                                 start=(h == 0), stop=(h == 1))
                            nc.vector.tensor_scalar(out=q_sb[:, ns], in0=qp,
                                                    scalar1=bq_sb, scalar2=None,
                                                    op0=OP.add)
                            kp = pp.tile([CQK, MT], f32, tag="kp")
                            for h in range(2):
                                nc.tensor.matmul(kp, wkT[:, h, :], xt[h][:, ns],
                                                 start=(h == 0), stop=(h == 1))
                            nc.vector.tensor_scalar(out=k_sb[:, ns], in0=kp,
                                                    scalar1=bk_sb, scalar2=None,
                                                    op0=OP.add)
                        for ni in range(NCH):
                            cs = slice(ni * 128, (ni + 1) * 128)
                            vp = vp_.tile([128, C], f32, tag="vp")
                            for h in range(2):
                                nc.tensor.matmul(vp, xt[h][:, cs], wvT[:, h, :],
                                                 start=(h == 0), stop=(h == 1))
                            nc.vector.tensor_tensor(out=vt_sb[:, ni, :], in0=vp,
                                                    in1=bv_row, op=OP.add)

                    # --- attention (chunk pairs, software-pipelined) ---
                    with tc.tile_pool(name="upsum", bufs=1, space="PSUM") as up, \
                         tc.tile_pool(name="dpsum", bufs=2, space="PSUM") as dpp, \
                         tc.tile_pool(name="tpsum", bufs=2, space="PSUM") as tpp:
                        for mt in range(NMT):
                            ms = slice(mt * MT, (mt + 1) * MT)
                            u0 = up.tile([128, MT], f32, tag="u0",
                                         name=f"u0_{b}_{mt}")
                            u1 = up.tile([128, MT], f32, tag="u1",
                                         name=f"u1_{b}_{mt}")
                            dp = dpp.tile([1, MT], f32, tag="dp",
                                          name=f"dp_{b}_{mt}")
                            tps, es = {}, {}

                            def t_stage(g):
                                tp = tpp.tile([128, 2, MT], f32, tag="tp",
                                              name=f"tp_{b}_{mt}_{g}")
                                for j in range(2):
                                    ni = 2 * g + j
                                    nc.tensor.matmul(
                                        tp[:, j, :],
                                        k_sb[:, ni * 128:(ni + 1) * 128],
                                        q_sb[:, ms], start=True, stop=True)
                                tps[g] = tp

                            def e_stage(g):
                                e = es[g] = epool.tile([128, 2, MT], bf16,
                                                       tag="e",
                                                       name=f"e_{b}_{mt}_{g}")
                                nc.scalar.activation(e, tps.pop(g), AF.Exp)

                            def u_stage(g):
                                e = es.pop(g)
                                for j in range(2):
                                    ni = 2 * g + j
                                    st = ni == 0
                                    sp = ni == NCH - 1
                                    ej = e[:, j, :]
                                    nc.tensor.matmul(u0, vt_sb[:, ni, 0:128],
                                                     ej, start=st, stop=sp)
                                    nc.tensor.matmul(u1, vt_sb[:, ni, 128:256],
                                                     ej, start=st, stop=sp)
                                    nc.tensor.matmul(dp, ones_k, ej,
                                                     start=st, stop=sp)

                            for g in range(NPAIR):
                                t_stage(g)
                                if g >= 1:
                                    e_stage(g - 1)
                                if g >= 2:
                                    u_stage(g - 2)
                            e_stage(NPAIR - 1)
                            u_stage(NPAIR - 2)
                            u_stage(NPAIR - 1)

                            r_sb = rpool.tile([1, MT], f32, tag="r")
                            nc.vector.reciprocal(r_sb, dp)
                            r128 = rpool.tile([128, MT], f32, tag="r128")
                            nc.gpsimd.partition_broadcast(r128, r_sb)
                            for h in range(2):
                                xr = xrpool.tile([128, MT], f32, tag="xr",
                                                 name=f"xr_{b}_{mt}_{h}")
                                nc.gpsimd.dma_start(
                                    out=xr,
                                    in_=xf_d[b, 128 * h:128 * (h + 1), ms])
                                u = u0 if h == 0 else u1
                                t1 = opool.tile([128, MT], f32, tag="t1")
                                nc.vector.scalar_tensor_tensor(
                                    out=t1, in0=u, scalar=gamma_b, in1=r128,
                                    op0=OP.mult, op1=OP.mult)
                                ot = opool.tile([128, MT], f32, tag="ot")
                                nc.vector.tensor_tensor(out=ot, in0=t1,
                                                        in1=xr, op=OP.add)
                                nc.gpsimd.dma_start(
                                    out=out_d[b, 128 * h:128 * (h + 1), ms],
                                    in_=ot)

            if repeat == 1:
                body()
            else:
                with tc.For_i(0, repeat, 1):
                    body()

    nc.finalize()
    return nc


_NC_CACHE = {}


def _get_nc():
    if "nc" not in _NC_CACHE:
        _NC_CACHE["nc"] = _build_nc()
    return _NC_CACHE["nc"]


def make_in_maps(inputs, wq, bq, wk, bk, wv, bv, gamma):
    import ml_dtypes
    bf16 = ml_dtypes.bfloat16

    x = np.ascontiguousarray(np.asarray(inputs, np.float32).reshape(B, C, N))
    xb = x.astype(bf16)
    wqT = np.ascontiguousarray(np.asarray(wq, np.float32).T).astype(bf16)
    wkT = np.ascontiguousarray(np.asarray(wk, np.float32).T).astype(bf16)
    wvT = np.ascontiguousarray(np.asarray(wv, np.float32).T).astype(bf16)
    bq = np.asarray(bq, np.float32)
    bk = np.asarray(bk, np.float32)
    bv = np.asarray(bv, np.float32)
    gamma = np.asarray(gamma, np.float32).reshape(1)

    in_maps = []
    for c in range(NCORES):
        sl = slice(c * BPC, (c + 1) * BPC)
        in_maps.append({
            "xb": xb[sl], "xf": x[sl],
            "wqT": wqT, "wkT": wkT, "wvT": wvT,
            "bq": bq, "bk": bk, "bv": bv, "gamma": gamma,
            "ones": np.ones(1, bf16),
        })
    return in_maps


def kernel(inputs, wq, bq, wk, bk, wv, bv, gamma):
    from concourse.bass_utils import run_bass_kernel_spmd

    nc = _get_nc()
    in_maps = make_in_maps(inputs, wq, bq, wk, bk, wv, bv, gamma)
    res = run_bass_kernel_spmd(nc, in_maps, core_ids=list(range(NCORES)))
    out = np.concatenate([res.results[c]["out"] for c in range(NCORES)], axis=0)
    return out.reshape(B, C, H, W)


# revision 15
# speedup vs baseline: 1.0556x; 1.0556x over previous
"""CNN attention (nn_CNNAttention_77979426226593) Trainium2 Bass kernel.

Data-parallel over batch: B=16 images -> 8 NeuronCores, 2 images per core.
Each core holds the full (small) conv1x1 weights and computes its local
N x N attention (N = H*W = 4096) independently.

Per image (C=256, N=4096, CQK=32):
  q = wq @ x + bq            [32, N]
  k = wk @ x + bk            [32, N]
  vt = x^T @ wv^T + bv       [N, 256]   (V transposed: needed as matmul lhsT)
  T[n, m] = k_n . q_m        (scores, transposed layout -> no transposes)
  E = exp(T)                 (softmax without max-subtraction: logits are
                              small by construction, exp fits fp32 easily)
  U[c, m] = sum_n vt[n, c] * E[n, m]
  d[m]    = sum_n E[n, m]    (ones-row matmul)
  out[c, m] = gamma * U[c, m] / d[m] + x[c, m]

Matmuls run in bf16 (measured ~3x faster per matmul than float32r and ~2.7x
faster than fp32 on this toolchain); accumulation is fp32 in PSUM, softmax
and normalization are fp32. The residual term x is added from a separate
fp32 copy, so when gamma == 0 the output equals the input bit-exactly.

The attention inner loop works on chunk PAIRS (one [128, 2, 512] PSUM score
tile -> a single 1024-wide exp on ScalarE) and is software-pipelined: the
score matmuls of pair g are issued before exp of pair g-1 and the U/d
matmuls of pair g-2, so the in-order PE queue never waits on ScalarE.
"""

import numpy as np

B, C, H, W = 16, 256, 64, 64
N = H * W          # 4096
CQK = 32
NCORES = 8
BPC = B // NCORES  # batches per core

MT = 512           # m tile (attention output columns per PSUM tile)
NMT = N // MT      # 8
NCH = N // 128     # 32 n-chunks (contraction for U)
NPAIR = NCH // 2   # 16 chunk pairs


def _build_nc(repeat=1):
    import contextlib
    import concourse.bacc as bacc
    import concourse.mybir as mybir
    import concourse.tile as tile
    import concourse.bass as bass

    f32 = mybir.dt.float32
    bf16 = mybir.dt.bfloat16
    AF = mybir.ActivationFunctionType
    OP = mybir.AluOpType

    nc = bacc.Bacc("TRN2", target_bir_lowering=False, debug=False,
                   num_devices=NCORES)

    xb_d = nc.dram_tensor("xb", [BPC, C, N], bf16, kind="ExternalInput")
    xf_d = nc.dram_tensor("xf", [BPC, C, N], f32, kind="ExternalInput")
    wqT_d = nc.dram_tensor("wqT", [C, CQK], bf16, kind="ExternalInput")
    wkT_d = nc.dram_tensor("wkT", [C, CQK], bf16, kind="ExternalInput")
    wvT_d = nc.dram_tensor("wvT", [C, C], bf16, kind="ExternalInput")
    bq_d = nc.dram_tensor("bq", [CQK], f32, kind="ExternalInput")
    bk_d = nc.dram_tensor("bk", [CQK], f32, kind="ExternalInput")
    bv_d = nc.dram_tensor("bv", [C], f32, kind="ExternalInput")
    gamma_d = nc.dram_tensor("gamma", [1], f32, kind="ExternalInput")
    ones_d = nc.dram_tensor("ones", [1], bf16, kind="ExternalInput")
    out_d = nc.dram_tensor("out", [BPC, C, N], f32, kind="ExternalOutput")

    def bcast_ap(handle, parts, free):
        # DRAM source AP replicated across `parts` partitions (step 0)
        return bass.AP(tensor=handle, offset=0, ap=[[0, parts], [1, free]])

    with tile.TileContext(nc) as tc:
        ctx = contextlib.ExitStack()
        with ctx:
            singles = ctx.enter_context(tc.tile_pool(name="singles", bufs=1))
            xpool = ctx.enter_context(tc.tile_pool(name="xpool", bufs=2))
            qkpool = ctx.enter_context(tc.tile_pool(name="qkpool", bufs=2))
            vtpool = ctx.enter_context(tc.tile_pool(name="vtpool", bufs=2))
            epool = ctx.enter_context(tc.tile_pool(name="epool", bufs=6))
            opool = ctx.enter_context(tc.tile_pool(name="opool", bufs=4))
            xrpool = ctx.enter_context(tc.tile_pool(name="xrpool", bufs=4))
            rpool = ctx.enter_context(tc.tile_pool(name="rpool", bufs=2))

            # --- constants / weights (once) ---
            wqT = singles.tile([C // 2, 2, CQK], bf16, tag="wqT")
            nc.gpsimd.dma_start(out=wqT, in_=wqT_d.ap().rearrange(
                "(t p) o -> p t o", p=128))
            wkT = singles.tile([C // 2, 2, CQK], bf16, tag="wkT")
            nc.gpsimd.dma_start(out=wkT, in_=wkT_d.ap().rearrange(
                "(t p) o -> p t o", p=128))
            wvT = singles.tile([C // 2, 2, C], bf16, tag="wvT")
            nc.gpsimd.dma_start(out=wvT, in_=wvT_d.ap().rearrange(
                "(t p) o -> p t o", p=128))
            bq_sb = singles.tile([128, 1], f32, tag="bq")
            nc.gpsimd.dma_start(out=bq_sb, in_=bass.AP(
                tensor=bq_d, offset=0, ap=[[0, 4], [1, CQK]]))
            bk_sb = singles.tile([128, 1], f32, tag="bk")
            nc.gpsimd.dma_start(out=bk_sb, in_=bass.AP(
                tensor=bk_d, offset=0, ap=[[0, 4], [1, CQK]]))
            bv_row = singles.tile([128, C], f32, tag="bvrow")
            nc.gpsimd.dma_start(out=bv_row, in_=bcast_ap(bv_d, 128, C))
            gamma_b = singles.tile([128, 1], f32, tag="gamma")
            nc.gpsimd.dma_start(out=gamma_b, in_=bcast_ap(gamma_d, 128, 1))
            ones_k = singles.tile([128, 1], bf16, tag="ones_k")
            nc.gpsimd.dma_start(out=ones_k, in_=bcast_ap(ones_d, 128, 1))

            def body():
                for b in range(BPC):
                    # --- load x (bf16 compute copy) ---
                    xt = [xpool.tile([128, N], bf16, tag=f"x{h}",
                                     name=f"xt{h}_{b}") for h in range(2)]
                    for h in range(2):
                        nc.gpsimd.dma_start(
                            out=xt[h], in_=xb_d[b, 128 * h:128 * (h + 1), :])

                    q_sb = qkpool.tile([128, N], bf16, tag="q")
                    k_sb = qkpool.tile([128, N], bf16, tag="k")
                    vt_sb = vtpool.tile([128, NCH, C], bf16, tag="vt")

                    # --- projections ---
                    with tc.tile_pool(name="ppsum", bufs=2, space="PSUM") as pp, \
                         tc.tile_pool(name="vpsum", bufs=2, space="PSUM") as vp_:
                        for nt in range(NMT):
                            ns = slice(nt * MT, (nt + 1) * MT)
                            qp = pp.tile([128, MT], f32, tag="qp")
                            for j in range(4):
                                for h in range(2):
                                    nc.tensor.matmul(
                                        qp[32 * j:32 * (j + 1), :],
                                        wqT[:, h, :], xt[h][:, ns],
                                        start=(h == 0), stop=(h == 1),
                                        tile_position=(0, 32 * j))
                            nc.vector.tensor_scalar(out=q_sb[:, ns], in0=qp,
                                                    scalar1=bq_sb, scalar2=None,
                                                    op0=OP.add)
                            kp = pp.tile([128, MT], f32, tag="kp")
                            for j in range(4):
                                for h in range(2):
                                    nc.tensor.matmul(
                                        kp[32 * j:32 * (j + 1), :],
                                        wkT[:, h, :], xt[h][:, ns],
                                        start=(h == 0), stop=(h == 1),
                                        tile_position=(0, 32 * j))
                            nc.vector.tensor_scalar(out=k_sb[:, ns], in0=kp,
                                                    scalar1=bk_sb, scalar2=None,
                                                    op0=OP.add)
                        for ni in range(NCH):
                            cs = slice(ni * 128, (ni + 1) * 128)
                            vp = vp_.tile([128, C], f32, tag="vp")
                            for h in range(2):
                                nc.tensor.matmul(vp, xt[h][:, cs], wvT[:, h, :],
                                                 start=(h == 0), stop=(h == 1))
                            nc.vector.tensor_tensor(out=vt_sb[:, ni, :], in0=vp,
                                                    in1=bv_row, op=OP.add)

                    # --- attention (chunk quads, software-pipelined) ---
                    with tc.tile_pool(name="upsum", bufs=1, space="PSUM") as up, \
                         tc.tile_pool(name="dpsum", bufs=2, space="PSUM") as dpp, \
                         tc.tile_pool(name="tpsum", bufs=1, space="PSUM") as tpp:
                        for mt in range(NMT):
                            ms = slice(mt * MT, (mt + 1) * MT)
                            u0 = up.tile([128, MT], f32, tag="u0",
                                         name=f"u0_{b}_{mt}")
                            u1 = up.tile([128, MT], f32, tag="u1",
                                         name=f"u1_{b}_{mt}")
                            dp = dpp.tile([1, MT], f32, tag="dp",
                                          name=f"dp_{b}_{mt}")
                            tps, es = {}, {}

                            def t_stage(g):
                                tp = tpp.tile([128, 4, MT], f32, tag="tp",
                                              name=f"tp_{b}_{mt}_{g}")
                                for j in range(4):
                                    ni = 4 * g + j
                                    nc.tensor.matmul(
                                        tp[:, j, :],
                                        k_sb[32 * j:32 * (j + 1),
                                             ni * 128:(ni + 1) * 128],
                                        q_sb[32 * j:32 * (j + 1), ms],
                                        start=True, stop=True,
                                        tile_position=(32 * j, 0))
                                tps[g] = tp

                            def e_stage(g):
                                e = es[g] = epool.tile([128, 4, MT], bf16,
                                                       tag="e",
                                                       name=f"e_{b}_{mt}_{g}")
                                nc.scalar.activation(e, tps.pop(g), AF.Exp)

                            def u_stage(g):
                                e = es.pop(g)
                                for j in range(4):
                                    ni = 4 * g + j
                                    st = ni == 0
                                    sp = ni == NCH - 1
                                    ej = e[:, j, :]
                                    nc.tensor.matmul(u0, vt_sb[:, ni, 0:128],
                                                     ej, start=st, stop=sp)
                                    nc.tensor.matmul(u1, vt_sb[:, ni, 128:256],
                                                     ej, start=st, stop=sp)
                                    nc.tensor.matmul(dp, ones_k, ej,
                                                     start=st, stop=sp)

                            NQ = NCH // 4
                            for g in range(NQ):
                                t_stage(g)
                                if g >= 1:
                                    e_stage(g - 1)
                                if g >= 2:
                                    u_stage(g - 2)
                            e_stage(NQ - 1)
                            u_stage(NQ - 2)
                            u_stage(NQ - 1)

                            # evict U accumulators so next m-tile's matmuls
                            # can reuse the PSUM banks immediately
                            uc = [opool.tile([128, MT], f32, tag=f"uc{h}",
                                             name=f"uc{h}_{b}_{mt}")
                                  for h in range(2)]
                            nc.vector.tensor_copy(uc[0], u0)
                            nc.vector.tensor_copy(uc[1], u1)
                            r_sb = rpool.tile([1, MT], f32, tag="r")
                            nc.vector.reciprocal(r_sb, dp)
                            r128 = rpool.tile([128, MT], f32, tag="r128")
                            nc.gpsimd.partition_broadcast(r128, r_sb)
                            for h in range(2):
                                xr = xrpool.tile([128, MT], f32, tag="xr",
                                                 name=f"xr_{b}_{mt}_{h}")
                                nc.gpsimd.dma_start(
                                    out=xr,
                                    in_=xf_d[b, 128 * h:128 * (h + 1), ms])
                                t1 = opool.tile([128, MT], f32, tag="t1")
                                nc.vector.scalar_tensor_tensor(
                                    out=t1, in0=uc[h], scalar=gamma_b, in1=r128,
                                    op0=OP.mult, op1=OP.mult)
                                ot = opool.tile([128, MT], f32, tag="ot")
                                nc.vector.tensor_tensor(out=ot, in0=t1,
                                                        in1=xr, op=OP.add)
                                nc.gpsimd.dma_start(
                                    out=out_d[b, 128 * h:128 * (h + 1), ms],
                                    in_=ot)

            if repeat == 1:
                body()
            else:
                with tc.For_i(0, repeat, 1):
                    body()

    nc.finalize()
    return nc


_NC_CACHE = {}


def _get_nc():
    if "nc" not in _NC_CACHE:
        _NC_CACHE["nc"] = _build_nc()
    return _NC_CACHE["nc"]


def make_in_maps(inputs, wq, bq, wk, bk, wv, bv, gamma):
    import ml_dtypes
    bf16 = ml_dtypes.bfloat16

    x = np.ascontiguousarray(np.asarray(inputs, np.float32).reshape(B, C, N))
    xb = x.astype(bf16)
    wqT = np.ascontiguousarray(np.asarray(wq, np.float32).T).astype(bf16)
    wkT = np.ascontiguousarray(np.asarray(wk, np.float32).T).astype(bf16)
    wvT = np.ascontiguousarray(np.asarray(wv, np.float32).T).astype(bf16)
    bq = np.asarray(bq, np.float32)
    bk = np.asarray(bk, np.float32)
    bv = np.asarray(bv, np.float32)
    gamma = np.asarray(gamma, np.float32).reshape(1)

    in_maps = []
    for c in range(NCORES):
        sl = slice(c * BPC, (c + 1) * BPC)
        in_maps.append({
            "xb": xb[sl], "xf": x[sl],
            "wqT": wqT, "wkT": wkT, "wvT": wvT,
            "bq": bq, "bk": bk, "bv": bv, "gamma": gamma,
            "ones": np.ones(1, bf16),
        })
    return in_maps


def kernel(inputs, wq, bq, wk, bk, wv, bv, gamma):
    from concourse.bass_utils import run_bass_kernel_spmd

    nc = _get_nc()
    in_maps = make_in_maps(inputs, wq, bq, wk, bk, wv, bv, gamma)
    res = run_bass_kernel_spmd(nc, in_maps, core_ids=list(range(NCORES)))
    out = np.concatenate([res.results[c]["out"] for c in range(NCORES)], axis=0)
    return out.reshape(B, C, H, W)
